# revision 1
# baseline (speedup 1.0000x reference)
"""Trainium2 Bass kernel for a 2-layer stacked bidirectional LSTM.

Problem (hardcoded): B=64, T=512, D=512, H=512, 2 BiLSTM layers,
Keras gate order [i, f, g, o], sigmoid recurrent activation, tanh cell
activation, merge_mode='concat'.

Sharding: 8 cores = 2 directions x 4 batch quarters (B'=16 per core).
Each core runs, for its (direction, quarter):
  phase 1: input projection zx0 = x @ W0 + b0      (big matmul, bf16)
  phase 2: layer-0 recurrence over T steps          (U0 stationary, bf16)
  phase 3: pairwise AllGather of layer-0 h sequences (fwd/bwd partners)
  phase 4: projection zx1 = [h0f | h0b] @ W1 + b1
  phase 5: layer-1 recurrence -> h1 sequence (f32 output)

Time-reversal for backward cores is handled with a per-core int flag and
register arithmetic on the recurrence's per-chunk DRAM indices, so all 8
cores run the identical SPMD program; all DRAM-resident sequences are in
TRUE time order.

Layouts (per core):
  xT   input [D, T*BQ] bf16, token = t*BQ + b  (true time)
  zxT  scratch [G/128=16, 128, T*BQ] bf16  (gate dim on partitions)
  h0T  scratch [T, H, BQ] bf16 (true time) -> AllGather -> [2, T, H, BQ]
  h1T  output [T, H, BQ] f32 (true time)

Recurrence state (SBUF): hT [128, H/128, BQ] bf16, c [128, H/128, BQ] f32.
Per step: z^T = U^T-accumulated PSUM (4 gate tiles, one PSUM bank each),
z-add with zx, sigmoid/tanh on ACT, cell update on DVE.
"""

import numpy as np
import ml_dtypes

import concourse.bass as bass
import concourse.mybir as mybir
import concourse.tile as tile
from concourse.bass import ds, ts
from concourse.bass_utils import run_bass_kernel_spmd
from concourse.expressions import smax

BF16 = mybir.dt.bfloat16
F32 = mybir.dt.float32
I32 = mybir.dt.int32
AF = mybir.ActivationFunctionType
ALU = mybir.AluOpType

# Problem dims (full size)
B_FULL, T_FULL, D_FULL, H_FULL = 64, 512, 512, 512
N_CORES = 8
N_Q = 4  # batch quarters; cores 2q (fwd) and 2q+1 (bwd) handle quarter q
CH = 8   # recurrence steps per For_i chunk

_MAXW = 1  # max sem-waits per instruction accepted by this walrus


def _fix_walrus_compat(nc):
    """Adapt Tile-emitted IR to the deployed walrus:
    - drop EVENT_SEMAPHORE_RANGE_CLEAR (InstISA) kernel-tail cleanup (only
      needed for NEFF re-execution with stale semaphores; each load starts
      from a clean state),
    - split instructions carrying more than _MAXW semaphore waits into
      leading single-wait NOPs (this walrus rejects multi-wait sync info).
    """
    n_split = n_drop = 0
    for bb in nc.main_func.blocks:
        insts = bb.instructions
        out = []
        for inst in insts:
            if isinstance(inst, mybir.InstISA):
                n_drop += 1
                continue
            si = inst.sync_info
            if si is not None and len(si.on_wait) > _MAXW:
                waits = list(si.on_wait)
                extra, keep = waits[:-_MAXW], waits[-_MAXW:]
                for w in extra:
                    nop = mybir.InstNoOp(
                        name=nc.get_next_instruction_name(), ins=[], outs=[])
                    nop.engine = inst.engine
                    nop.sync_info = mybir.SyncInfo(on_wait=[w], on_update=[])
                    out.append(nop)
                    n_split += 1
                inst.sync_info = mybir.SyncInfo(
                    on_wait=keep, on_update=list(si.on_update))
            out.append(inst)
        insts[:] = out
    return n_drop, n_split


def build_program(T=T_FULL, BQ=B_FULL // N_Q, D=D_FULL, H=H_FULL,
                  single_core=False):
    """Build the SPMD bass program (identical for all 8 cores).

    single_core=True replaces the AllGather with local DMA copies (for
    TimelineSim cost analysis only — data is wrong for the partner block).
    """
    G = 4 * H
    KD0 = D // 128          # k-chunks layer-0 projection
    KD1 = 2 * H // 128      # k-chunks layer-1 projection
    KH = H // 128           # k-chunks recurrence / h storage
    MC = G // 128           # m-chunks of gate dim
    MG = MC // 4            # m-chunks per gate
    TOK = T * BQ            # tokens per core
    TW = min(512, TOK)      # proj token-tile width
    NTOKC = TOK // TW       # proj token tiles
    NT = T // CH            # recurrence chunks
    assert T % CH == 0 and TOK % TW == 0 and D % 128 == 0 and H % 128 == 0

    nc = bass.Bass("TRN2", target_bir_lowering=False, debug=False,
                   num_devices=1 if single_core else N_CORES)

    # ---- I/O ----
    xT = nc.dram_tensor("xT", [D, TOK], BF16, kind="ExternalInput")
    flagf = nc.dram_tensor("flagf", [1, 1], F32, kind="ExternalInput")
    w0 = nc.dram_tensor("w0", [D, G], BF16, kind="ExternalInput")
    u0 = nc.dram_tensor("u0", [H, G], BF16, kind="ExternalInput")
    b0 = nc.dram_tensor("b0", [G], F32, kind="ExternalInput")
    w1 = nc.dram_tensor("w1", [2 * H, G], BF16, kind="ExternalInput")
    u1 = nc.dram_tensor("u1", [H, G], BF16, kind="ExternalInput")
    b1 = nc.dram_tensor("b1", [G], F32, kind="ExternalInput")
    flag = nc.dram_tensor("flag", [1, 1], I32, kind="ExternalInput")
    h1T = nc.dram_tensor("h1T", [T, H, BQ], F32, kind="ExternalOutput")

    groups = [[2 * q, 2 * q + 1] for q in range(N_Q)]

    with tile.TileContext(nc) as tc:
        # ---------------- persistent pools ----------------
        consts = tc.alloc_tile_pool(name="consts", bufs=1)
        dram = tc.alloc_tile_pool(name="dram", bufs=1, space="DRAM")

        # weights / biases resident in SBUF for the whole kernel
        w0_sb = consts.tile([128, KD0, G], BF16)
        nc.sync.dma_start(w0_sb, w0.ap().rearrange("(k p) g -> p k g", p=128))
        u0_sb = consts.tile([128, KH, G], BF16)
        nc.sync.dma_start(u0_sb, u0.ap().rearrange("(k p) g -> p k g", p=128))
        w1_sb = consts.tile([128, KD1, G], BF16)
        nc.sync.dma_start(w1_sb, w1.ap().rearrange("(k p) g -> p k g", p=128))
        u1_sb = consts.tile([128, KH, G], BF16)
        nc.sync.dma_start(u1_sb, u1.ap().rearrange("(k p) g -> p k g", p=128))
        b0_sb = consts.tile([128, MC], F32)
        nc.sync.dma_start(b0_sb, b0.ap().rearrange("(m p) -> p m", p=128))
        b1_sb = consts.tile([128, MC], F32)
        nc.sync.dma_start(b1_sb, b1.ap().rearrange("(m p) -> p m", p=128))
        flag_sb = consts.tile([1, 1], I32)
        nc.sync.dma_start(flag_sb, flag.ap())
        # broadcast flag as f32 per-partition scalar F (and 1-F) for the
        # data-driven time-flip selects
        F_bc = consts.tile([128, 1], F32)
        nc.sync.dma_start(
            F_bc,
            bass.AP(tensor=flagf, offset=0, ap=[[0, 128], [1, 1]]))
        Fc_bc = consts.tile([128, 1], F32)
        nc.vector.memset(Fc_bc, 1.0)
        nc.vector.tensor_tensor(Fc_bc, Fc_bc, F_bc, ALU.subtract)

        # DRAM scratch
        zxT0 = dram.tile([MC, 128, TOK], BF16)
        zxT1 = dram.tile([MC, 128, TOK], BF16)
        ag_in = dram.tile([T, H, BQ], BF16)
        ag_out = dram.tile([2 * T, H, BQ], BF16)

        fv = nc.values_load(flag_sb[0:1, 0:1], min_val=0, max_val=1)

        # ---------------- phase: projection ----------------
        def projection(src_kind, w_sb, b_sb, zxT, KD):
            with tc.tile_pool(name=f"proj_x_{src_kind}", bufs=2) as xpool, \
                 tc.tile_pool(name=f"proj_ps_{src_kind}", bufs=4,
                              space="PSUM") as pspool, \
                 tc.tile_pool(name=f"proj_ev_{src_kind}", bufs=4) as evpool:
                for tokc in range(NTOKC):
                    x_sb = xpool.tile([128, KD, TW], BF16, tag="xsb")
                    if src_kind == "xT":
                        nc.sync.dma_start(
                            x_sb,
                            xT.ap().rearrange("(k p) t -> p k t", p=128)[
                                :, :, ts(tokc, TW)])
                    else:
                        # ag_out [2T, H, BQ]: feature f = dir*H + 128*hk + p
                        # token tile tokc covers t in [tokc*TT, (tokc+1)*TT)
                        TT = TW // BQ
                        src = ag_out.rearrange(
                            "(dir t) (hk p) b -> p dir hk t b", dir=2, p=128)
                        for kk in range(KD):
                            nc.sync.dma_start(
                                x_sb[:, kk, :],
                                src[:, kk // (KD // 2), kk % (KD // 2),
                                    ts(tokc, TT), :])
                    for m in range(MC):
                        psum = pspool.tile([128, TW], F32, tag="pps")
                        for k in range(KD):
                            nc.tensor.matmul(
                                psum, w_sb[:, k, ts(m, 128)], x_sb[:, k, :],
                                start=(k == 0), stop=(k == KD - 1))
                        ev = evpool.tile([128, TW], BF16, tag="pev")
                        nc.scalar.activation(ev, psum, AF.Identity,
                                             bias=b_sb[:, m:m + 1])
                        nc.sync.dma_start(zxT[m][:, ts(tokc, TW)], ev)

        # ---------------- phase: recurrence ----------------
        def recurrence(u_sb, zxT, layer):
            state = tc.alloc_tile_pool(name=f"state{layer}", bufs=1)
            h_sb = state.tile([128, KH, BQ], BF16, name=f"hsb{layer}")
            c_sb = state.tile([128, KH, BQ], F32, name=f"csb{layer}")
            nc.vector.memset(h_sb, 0.0)
            nc.vector.memset(c_sb, 0.0)

            zx_r = zxT.rearrange("m p tok -> p m tok")
            if layer == 0:
                hT_r = ag_in.rearrange("t (k p) b -> p t k b", p=128)
            else:
                hT_r = h1T.ap().rearrange("t (k p) b -> p t k b", p=128)

            with tc.tile_pool(name=f"zx{layer}", bufs=2) as zxpool, \
                 tc.tile_pool(name=f"hck{layer}", bufs=2) as hckpool, \
                 tc.tile_pool(name=f"zg{layer}", bufs=2) as zgpool, \
                 tc.tile_pool(name=f"ps_i{layer}", bufs=2,
                              space="PSUM") as ps_i, \
                 tc.tile_pool(name=f"ps_f{layer}", bufs=2,
                              space="PSUM") as ps_f, \
                 tc.tile_pool(name=f"ps_g{layer}", bufs=2,
                              space="PSUM") as ps_g, \
                 tc.tile_pool(name=f"ps_o{layer}", bufs=2,
                              space="PSUM") as ps_o:
                gate_pools = [ps_i, ps_f, ps_g, ps_o]
                with tc.For_i(0, NT, 1) as cc:
                    # true-time chunk index (flipped for bwd cores):
                    # fwd: cc ; bwd: NT-1-cc   via smax(cc - K, K - cc)
                    K = fv * (NT - 1)
                    tcix = nc.s_assert_within(smax(cc - K, K - cc), 0, NT - 1)
                    zx_sb = zxpool.tile([128, MC, CH * BQ], BF16, tag="zxc")
                    nc.sync.dma_start(
                        zx_sb, zx_r[:, :, ds(tcix * (CH * BQ), CH * BQ)])
                    hdt = F32 if layer == 1 else BF16
                    h_ck = hckpool.tile([128, CH, KH, BQ], hdt, tag="hck")
                    h_ckT = hckpool.tile([128, CH, KH, BQ], hdt, tag="hckT")
                    for j in range(CH):
                        psg = [gate_pools[g].tile([128, MG, BQ], F32,
                                                  tag=f"psg{g}",
                                                  name=f"psg{g}")
                               for g in range(4)]
                        z16 = zgpool.tile([128, MC, BQ], F32, tag="z16")
                        g16 = zgpool.tile([128, MC, BQ], F32, tag="g16")
                        # local-time zx slice via data-driven select:
                        # zxj = (1-F)*zx[j] + F*zx[CH-1-j]
                        zxj = zgpool.tile([128, MC, BQ], F32, tag="zxj")
                        tmpz = zgpool.tile([128, MC, BQ], F32, tag="tmpz")
                        nc.vector.tensor_scalar_mul(
                            tmpz, zx_sb[:, :, ts(CH - 1 - j, BQ)], F_bc)
                        nc.vector.scalar_tensor_tensor(
                            zxj, zx_sb[:, :, ts(j, BQ)], Fc_bc, tmpz,
                            ALU.mult, ALU.add)
                        for g in range(4):
                            for mm in range(MG):
                                m = g * MG + mm
                                for k in range(KH):
                                    nc.tensor.matmul(
                                        psg[g][:, mm, :],
                                        u_sb[:, k, ts(m, 128)],
                                        h_sb[:, k, :],
                                        start=(k == 0), stop=(k == KH - 1))
                            nc.vector.tensor_tensor(
                                z16[:, ts(g, MG), :], psg[g],
                                zxj[:, ts(g, MG), :], ALU.add)
                        # activations: sigmoid(i,f), tanh(g), sigmoid(o)
                        nc.scalar.activation(g16[:, 0:2 * MG, :],
                                             z16[:, 0:2 * MG, :], AF.Sigmoid)
                        nc.scalar.activation(g16[:, 2 * MG:3 * MG, :],
                                             z16[:, 2 * MG:3 * MG, :], AF.Tanh)
                        nc.scalar.activation(g16[:, 3 * MG:4 * MG, :],
                                             z16[:, 3 * MG:4 * MG, :],
                                             AF.Sigmoid)
                        ig = zgpool.tile([128, MG, BQ], F32, tag="ig")
                        fc = zgpool.tile([128, MG, BQ], F32, tag="fc")
                        nc.vector.tensor_tensor(ig, g16[:, 0:MG, :],
                                                g16[:, 2 * MG:3 * MG, :],
                                                ALU.mult)
                        nc.vector.tensor_tensor(fc, g16[:, MG:2 * MG, :],
                                                c_sb, ALU.mult)
                        nc.vector.tensor_tensor(c_sb, ig, fc, ALU.add)
                        th = zgpool.tile([128, MG, BQ], F32, tag="th")
                        nc.scalar.activation(th, c_sb, AF.Tanh)
                        # h = o * tanh(c): bf16 state copy for next matmul
                        nc.vector.tensor_tensor(h_sb, g16[:, 3 * MG:, :], th,
                                                ALU.mult)
                        # store into h chunk at local slot (off crit path)
                        if layer == 0:
                            nc.scalar.copy(h_ck[:, j, :, :], h_sb)
                        else:
                            nc.vector.tensor_tensor(
                                h_ck[:, j, :, :],
                                g16[:, 3 * MG:, :], th, ALU.mult)
                    # reorder chunk local->true time on gpsimd (idle engine):
                    # h_ckT[s] = (1-F)*h_ck[s] + F*h_ck[CH-1-s]
                    for s in range(CH):
                        tsel = zgpool.tile([128, KH, BQ], hdt, tag="tsel")
                        nc.vector.tensor_scalar_mul(
                            tsel, h_ck[:, CH - 1 - s, :, :], F_bc)
                        nc.vector.scalar_tensor_tensor(
                            h_ckT[:, s, :, :], h_ck[:, s, :, :], Fc_bc, tsel,
                            ALU.mult, ALU.add)
                    nc.sync.dma_start(
                        hT_r[:, ds(tcix * CH, CH), :, :], h_ckT)
            state.release()

        projection("xT", w0_sb, b0_sb, zxT0, KD0)
        recurrence(u0_sb, zxT0, 0)
        if single_core:
            ag_v = ag_out.rearrange("(dir t) h b -> dir t h b", dir=2)
            nc.sync.dma_start(ag_v[0], ag_in)
            nc.sync.dma_start(ag_v[1], ag_in)
        else:
            nc.gpsimd.collective_compute(
                "AllGather", ALU.bypass, replica_groups=groups,
                ins=[ag_in.opt()], outs=[ag_out.opt()])
        projection("ag", w1_sb, b1_sb, zxT1, KD1)
        recurrence(u1_sb, zxT1, 1)

        dram.release()
        consts.release()

    _fix_walrus_compat(nc)
    return nc


def _prep_core_inputs(x, W0f, U0f, b0f, W0b, U0b, b0b,
                      W1f, U1f, b1f, W1b, U1b, b1b, T, BQ):
    """Host-side sharding: returns list of 8 input dicts (core = 2q+dir)."""
    bf = ml_dtypes.bfloat16
    in_maps = []
    Wd = {0: (W0f, U0f, b0f, W1f, U1f, b1f),
          1: (W0b, U0b, b0b, W1b, U1b, b1b)}
    for q in range(N_Q):
        xq = x[q * BQ:(q + 1) * BQ]              # [BQ, T, D]
        # xT [D, T*BQ], tok = t*BQ + b, true time for both directions
        xT = np.ascontiguousarray(
            xq.transpose(2, 1, 0).reshape(x.shape[2], T * BQ)).astype(bf)
        for d in range(2):
            W0, U0, b0, W1, U1, b1 = Wd[d]
            in_maps.append({
                "xT": xT,
                "w0": W0.astype(bf), "u0": U0.astype(bf),
                "b0": b0.astype(np.float32),
                "w1": W1.astype(bf), "u1": U1.astype(bf),
                "b1": b1.astype(np.float32),
                "flag": np.array([[d]], dtype=np.int32),
                "flagf": np.array([[d]], dtype=np.float32),
            })
    return in_maps


def kernel(x, W0f, U0f, b0f, W0b, U0b, b0b,
           W1f, U1f, b1f, W1b, U1b, b1b):
    x = np.asarray(x, dtype=np.float32)
    B, T, D = x.shape
    H = U0f.shape[0]
    BQ = B // N_Q
    nc = build_program(T=T, BQ=BQ, D=D, H=H)
    in_maps = _prep_core_inputs(
        np.asarray(x), np.asarray(W0f), np.asarray(U0f), np.asarray(b0f),
        np.asarray(W0b), np.asarray(U0b), np.asarray(b0b),
        np.asarray(W1f), np.asarray(U1f), np.asarray(b1f),
        np.asarray(W1b), np.asarray(U1b), np.asarray(b1b), T, BQ)
    res = run_bass_kernel_spmd(nc, in_maps, list(range(N_CORES)))
    out = np.empty((B, T, 2 * H), dtype=np.float32)
    for q in range(N_Q):
        for d in range(2):
            h1T = res.results[2 * q + d]["h1T"]   # [T, H, BQ] true time
            out[q * BQ:(q + 1) * BQ, :, d * H:(d + 1) * H] = \
                h1T.transpose(2, 0, 1)
    return out



# revision 9
# speedup vs baseline: 1.6699x; 1.6699x over previous
"""Trainium2 Bass kernel for a 2-layer stacked bidirectional LSTM.

Problem (hardcoded): B=64, T=512, D=512, H=512, 2 BiLSTM layers,
Keras gate order [i, f, g, o], sigmoid recurrent activation, tanh cell
activation, merge_mode='concat'.

Sharding: 8 cores = 2 directions x 4 batch quarters (B'=16 per core).

v2 design (all cores run the identical SPMD program, ALL in LOCAL time):
  - Host pre-reverses x in time for backward cores, packs weights with the
    gate order permuted to [i, f, o, g] (so sigmoid gates are contiguous),
    swaps W1's row blocks per direction (own-direction rows first), and
    un-reverses the output for backward cores.
  - Recurrence is fully unrolled in python (no hardware loops): 16 chunks
    of 32 steps per layer. zx (input projections) live only in SBUF; the
    projection matmuls for chunk cc+1 are interleaved between the
    recurrence steps of chunk cc so the PE stays dense (HAM warm) and has
    no DRAM zx roundtrip.
  - Gate PSUM: one bank holds the 12 sigmoid-gate m-chunks, another the 4
    tanh m-chunks (m-outer, k-inner accumulation groups). The sigmoid
    z-add + activation overlap the tanh-block matmuls (different banks).
  - Per step tail (critical path): tanh-z-add -> tanh -> ig -> c -> tanh(c)
    -> h, with f*c computed early in parallel.
  - Layer 0 h sequence goes to DRAM (local time), pairwise AllGather with
    the partner core, layer-1 projection reads own half straight and the
    partner half chunk-reversed (static) + step-reversed via gpsimd
    copies (negative-stride DMAs are rejected by walrus).
"""

import numpy as np
import ml_dtypes

import concourse.bass as bass
import concourse.mybir as mybir
import concourse.tile as tile
from concourse.bass import ds, ts
from concourse.bass_utils import run_bass_kernel_spmd

BF16 = mybir.dt.bfloat16
F32 = mybir.dt.float32
I32 = mybir.dt.int32
AF = mybir.ActivationFunctionType
ALU = mybir.AluOpType

# Problem dims (full size)
B_FULL, T_FULL, D_FULL, H_FULL = 64, 512, 512, 512
N_CORES = 8
N_Q = 4   # batch quarters; cores 2q (fwd) and 2q+1 (bwd) handle quarter q
CH = 32   # recurrence steps per chunk (CH*BQ = 512 tokens per chunk)

_MAXW = 1  # max sem-waits per instruction accepted by this walrus


def _fix_walrus_compat(nc):
    """Adapt Tile-emitted IR to the deployed walrus:
    - drop EVENT_SEMAPHORE_RANGE_CLEAR (InstISA) kernel-tail cleanup,
    - split instructions carrying more than _MAXW semaphore waits into
      leading single-wait NOPs.
    """
    n_split = n_drop = 0
    for bb in nc.main_func.blocks:
        insts = bb.instructions
        out = []
        for inst in insts:
            if isinstance(inst, mybir.InstISA):
                n_drop += 1
                continue
            si = inst.sync_info
            if si is not None and len(si.on_wait) > _MAXW:
                waits = list(si.on_wait)
                extra, keep = waits[:-_MAXW], waits[-_MAXW:]
                for w in extra:
                    nop = mybir.InstNoOp(
                        name=nc.get_next_instruction_name(), ins=[], outs=[])
                    nop.engine = inst.engine
                    nop.sync_info = mybir.SyncInfo(on_wait=[w], on_update=[])
                    out.append(nop)
                    n_split += 1
                inst.sync_info = mybir.SyncInfo(
                    on_wait=keep, on_update=list(si.on_update))
            out.append(inst)
        insts[:] = out
    return n_drop, n_split


def build_program(T=T_FULL, BQ=B_FULL // N_Q, D=D_FULL, H=H_FULL,
                  single_core=False):
    G = 4 * H
    KD0 = D // 128           # k-chunks, layer-0 projection
    KD1 = 2 * H // 128       # k-chunks, layer-1 projection
    KH = H // 128            # k-chunks, recurrence
    MC = G // 128            # m-chunks of the gate dim
    MS = 3 * (MC // 4)       # sigmoid m-chunks (i, f, o)
    MT = MC // 4             # tanh m-chunks (g)
    NT = T // CH             # chunks per layer
    TW = CH * BQ             # tokens per chunk (= proj tile width)
    FH = KH * CH * BQ        # flattened h-chunk free size
    assert T % CH == 0 and D % 128 == 0 and H % 128 == 0

    nc = bass.Bass("TRN2", target_bir_lowering=False, debug=False,
                   num_devices=1 if single_core else N_CORES)

    # ---- I/O (all host-packed, local time, gate order [i,f,o,g]) ----
    x_in = nc.dram_tensor("x_in", [NT, 128, KD0 * TW], BF16,
                          kind="ExternalInput")
    w0 = nc.dram_tensor("w0", [128, KD0 * G], BF16, kind="ExternalInput")
    u0 = nc.dram_tensor("u0", [128, KH * G], BF16, kind="ExternalInput")
    b0 = nc.dram_tensor("b0", [128, MC], F32, kind="ExternalInput")
    w1 = nc.dram_tensor("w1", [128, KD1 * G], BF16, kind="ExternalInput")
    u1 = nc.dram_tensor("u1", [128, KH * G], BF16, kind="ExternalInput")
    b1 = nc.dram_tensor("b1", [128, MC], F32, kind="ExternalInput")
    pflag = nc.dram_tensor("pflag", [1, 1], I32, kind="ExternalInput")
    h1out = nc.dram_tensor("h1out", [NT, 128, FH], BF16,
                           kind="ExternalOutput")

    groups = [[2 * q, 2 * q + 1] for q in range(N_Q)]

    with tile.TileContext(nc) as tc:
        consts = tc.alloc_tile_pool(name="consts", bufs=1)
        dram = tc.alloc_tile_pool(name="dram", bufs=1, space="DRAM")

        # weights / biases resident in SBUF for the whole kernel
        w0_sb = consts.tile([128, KD0, G], BF16)
        nc.sync.dma_start(w0_sb, w0.ap())
        u0_sb = consts.tile([128, KH, G], BF16)
        nc.sync.dma_start(u0_sb, u0.ap())
        w1_sb = consts.tile([128, KD1, G], BF16)
        nc.sync.dma_start(w1_sb, w1.ap())
        u1_sb = consts.tile([128, KH, G], BF16)
        nc.sync.dma_start(u1_sb, u1.ap())
        b0_sb = consts.tile([128, MC], F32)
        nc.sync.dma_start(b0_sb, b0.ap())
        b1_sb = consts.tile([128, MC], F32)
        nc.sync.dma_start(b1_sb, b1.ap())
        pflag_sb = consts.tile([1, 1], I32)
        nc.sync.dma_start(pflag_sb, pflag.ap())
        zero_h = consts.tile([128, KH, BQ], BF16)
        nc.vector.memset(zero_h, 0.0)
        # cell state + tanh(g) scratch per layer: X[:, 0:KH] = tanh(g),
        # X[:, KH:2KH] = c
        X0 = consts.tile([128, 2 * KH, BQ], F32, name="X0")
        X1 = consts.tile([128, 2 * KH, BQ], F32, name="X1")
        nc.vector.memset(X0, 0.0)
        nc.vector.memset(X1, 0.0)

        # DRAM scratch: layer-0 h sequence (local time) + AllGather result
        # + partner's block (one dynamic-offset copy out of ag_out)
        h0loc = dram.tile([NT, 128, FH], BF16)
        ag_out = dram.tile([2 * NT, 128, FH], BF16)
        part = dram.tile([NT, 128, FH], BF16)

        fvp = nc.values_load(pflag_sb[0:1, 0:1], min_val=0, max_val=1)

        # ---------------- pools ----------------
        xpool = tc.alloc_tile_pool(name="xpool", bufs=2)
        zxpool = tc.alloc_tile_pool(name="zxpool", bufs=2)
        hckpool = tc.alloc_tile_pool(name="hckpool", bufs=2)
        tailpool = tc.alloc_tile_pool(name="tailpool", bufs=2)
        ps_sig = tc.alloc_tile_pool(name="ps_sig", bufs=1, space="PSUM")
        ps_tanh = tc.alloc_tile_pool(name="ps_tanh", bufs=1, space="PSUM")
        pp = tc.alloc_tile_pool(name="pp", bufs=2, space="PSUM")

        psig = ps_sig.tile([128, MS, BQ], F32, name="psig")
        ptanh = ps_tanh.tile([128, MT, BQ], F32, name="ptanh")

        # ---------------- helpers ----------------
        def load_x0(cc, xs):
            """x tile for layer-0 projection of chunk cc."""
            nc.sync.dma_start(
                xs, x_in.ap()[cc].rearrange("p (k t) -> p k t", k=KD0))

        def proj_group(m, w_sb, x_sb, KD, zx_sb, b_sb):
            """One projection m-group: 4|8 matmuls (N=TW) + biased copy."""
            ps = pp.tile([128, TW], F32, tag="pp")
            for k in range(KD):
                nc.tensor.matmul(ps, w_sb[:, k, ts(m, 128)], x_sb[:, k, :],
                                 start=(k == 0), stop=(k == KD - 1))
            nc.scalar.activation(
                zx_sb[:, :, m, :],
                ps.rearrange("p (j b) -> p j b", j=CH),
                AF.Identity, bias=b_sb[:, m:m + 1])

        def load_x1(cc, xs, pt):
            """Layer-1 proj inputs for chunk cc: own half straight from
            h0loc[cc]; partner half = part[NT-1-cc] (chunk-reversed)."""
            nc.sync.dma_start(
                xs[:, 0:KH, :],
                h0loc[cc].rearrange("p (k t) -> p k t", k=KH))
            nc.sync.dma_start(pt[:, 0, :], part[NT - 1 - cc])

        def reverse_steps(pt, xs, j_lo, j_hi):
            """Step-reverse partner h chunk into xs[:, KH:2KH, :]."""
            ptv = pt.rearrange("p one (k j b) -> p one k j b", k=KH, j=CH)
            for j in range(j_lo, j_hi):
                nc.gpsimd.tensor_copy(
                    xs[:, KH:2 * KH, ts(CH - 1 - j, BQ)],
                    ptv[:, 0, :, j, :])

        # ---------------- one recurrence step ----------------
        def step(u_sb, zx_sb, j, rhs_tile, rhs_j, h_ck, X):
            """rhs = rhs_tile[:, k, rhs_j, :] ([128,KH,CH,BQ]) or zero_h
            ([128,KH,BQ]) when rhs_j is None."""
            def rhs(k):
                if rhs_j is None:
                    return rhs_tile[:, k, :]
                return rhs_tile[:, k, rhs_j, :]

            for m in range(MC):
                dst = psig[:, m, :] if m < MS else ptanh[:, m - MS, :]
                for k in range(KH):
                    nc.tensor.matmul(dst, u_sb[:, k, ts(m, 128)], rhs(k),
                                     start=(k == 0), stop=(k == KH - 1))
                if m == MS - 1:
                    # sigmoid block complete: z-add + sigmoid overlap the
                    # tanh-block matmuls (different PSUM banks)
                    Zs = tailpool.tile([128, MS, BQ], F32, tag="Zs")
                    S = tailpool.tile([128, MS, BQ], F32, tag="S")
                    nc.vector.tensor_tensor(
                        Zs, psig, zx_sb[:, j, 0:MS, :], ALU.add)
                    nc.scalar.activation(S, Zs, AF.Sigmoid)
            # tanh block
            Zt = tailpool.tile([128, MT, BQ], F32, tag="Zt")
            nc.vector.tensor_tensor(
                Zt, ptanh, zx_sb[:, j, MS:MC, :], ALU.add)
            # f*c early (off the critical chain): fc = S[f] * X[c]
            fc = tailpool.tile([128, KH, BQ], F32, tag="fc")
            nc.vector.tensor_tensor(
                fc, S[:, KH:2 * KH, :], X[:, KH:2 * KH, :], ALU.mult)
            # critical chain: tanh(g) -> ig -> c -> tanh(c) -> h
            nc.scalar.activation(X[:, 0:KH, :], Zt, AF.Tanh)
            ig = tailpool.tile([128, KH, BQ], F32, tag="ig")
            nc.vector.tensor_tensor(
                ig, S[:, 0:KH, :], X[:, 0:KH, :], ALU.mult)
            nc.vector.tensor_tensor(X[:, KH:2 * KH, :], ig, fc, ALU.add)
            th = tailpool.tile([128, KH, BQ], F32, tag="th")
            nc.scalar.activation(th, X[:, KH:2 * KH, :], AF.Tanh)
            nc.vector.tensor_tensor(
                h_ck[:, :, j, :], S[:, 2 * KH:3 * KH, :], th, ALU.mult)

        # ================= layer 0 =================
        x_sb = {}
        x_sb[0] = xpool.tile([128, KD0, TW], BF16, tag="x0", name="xsb0")
        load_x0(0, x_sb[0])
        zx = {}
        zx[0] = zxpool.tile([128, CH, MC, BQ], BF16, tag="zx", name="zx0")
        for m in range(MC):
            proj_group(m, w0_sb, x_sb[0], KD0, zx[0], b0_sb)
        x_sb[1] = xpool.tile([128, KD0, TW], BF16, tag="x0", name="xsb1")
        load_x0(1, x_sb[1])

        h_ck_prev = None
        for cc in range(NT):
            h_ck = hckpool.tile([128, KH, CH, BQ], BF16, tag="hck0")
            if cc + 1 < NT:
                zx[cc + 1] = zxpool.tile([128, CH, MC, BQ], BF16, tag="zx", name=f"zx{cc+1}")
            for j in range(CH):
                if j == 0:
                    if cc == 0:
                        step(u0_sb, zx[cc], j, zero_h, None, h_ck, X0)
                    else:
                        step(u0_sb, zx[cc], j, h_ck_prev, CH - 1, h_ck, X0)
                else:
                    step(u0_sb, zx[cc], j, h_ck, j - 1, h_ck, X0)
                if j == 1 and cc + 2 < NT:
                    x_sb[cc + 2] = xpool.tile([128, KD0, TW], BF16, tag="x0", name=f"xsb{cc+2}")
                    load_x0(cc + 2, x_sb[cc + 2])
                if j % 2 == 0 and cc + 1 < NT:
                    proj_group(j // 2, w0_sb, x_sb[cc + 1], KD0,
                               zx[cc + 1], b0_sb)
            nc.sync.dma_start(h0loc[cc], h_ck.rearrange("p k j b -> p (k j b)"))
            h_ck_prev = h_ck

        # ================= AllGather =================
        if single_core:
            nc.sync.dma_start(ag_out[0:NT], h0loc)
            nc.sync.dma_start(ag_out[NT:2 * NT], h0loc)
        else:
            nc.gpsimd.collective_compute(
                "AllGather", ALU.bypass, replica_groups=groups,
                ins=[h0loc.opt()], outs=[ag_out.opt()])
        # copy the partner's whole block out of ag_out with a single
        # dynamic-offset DMA; everything downstream is then static
        poff = nc.s_assert_within(fvp * (NT * 128 * FH), 0, NT * 128 * FH)
        apg = ag_out[0]
        nc.sync.dma_start(
            part.rearrange("c p f -> (c p) f"),
            bass.AP(tensor=apg.tensor, offset=apg.offset + poff,
                    ap=[[FH, NT * 128], [1, FH]]))

        # ================= layer 1 =================
        x1 = {}
        pt = {}
        x1[0] = xpool.tile([128, KD1, TW], BF16, tag="x1", name="x1_0")
        pt[0] = xpool.tile([128, 1, FH], BF16, tag="pt", name="pt0")
        load_x1(0, x1[0], pt[0])
        reverse_steps(pt[0], x1[0], 0, CH)
        zx1 = {}
        zx1[0] = zxpool.tile([128, CH, MC, BQ], BF16, tag="zx", name="zx1_0")
        for m in range(MC):
            proj_group(m, w1_sb, x1[0], KD1, zx1[0], b1_sb)

        h_ck_prev = None
        for cc in range(NT):
            h_ck = hckpool.tile([128, KH, CH, BQ], BF16, tag="hck1")
            if cc + 1 < NT:
                zx1[cc + 1] = zxpool.tile([128, CH, MC, BQ], BF16, tag="zx", name=f"zx1_{cc+1}")
            for j in range(CH):
                if j == 0:
                    if cc == 0:
                        step(u1_sb, zx1[cc], j, zero_h, None, h_ck, X1)
                    else:
                        step(u1_sb, zx1[cc], j, h_ck_prev, CH - 1, h_ck, X1)
                else:
                    step(u1_sb, zx1[cc], j, h_ck, j - 1, h_ck, X1)
                if cc + 1 < NT:
                    if j == 0:
                        x1[cc + 1] = xpool.tile([128, KD1, TW], BF16,
                                                tag="x1", name=f"x1_{cc+1}")
                        pt[cc + 1] = xpool.tile([128, 1, FH], BF16, tag="pt", name=f"pt{cc+1}")
                        load_x1(cc + 1, x1[cc + 1], pt[cc + 1])
                    if 2 <= j < 10:
                        reverse_steps(pt[cc + 1], x1[cc + 1],
                                      (j - 2) * 4, (j - 1) * 4)
                    if 10 <= j < 26:
                        proj_group(j - 10, w1_sb, x1[cc + 1], KD1,
                                   zx1[cc + 1], b1_sb)
            nc.sync.dma_start(h1out.ap()[cc],
                              h_ck.rearrange("p k j b -> p (k j b)"))
            h_ck_prev = h_ck

        for p in (pp, ps_tanh, ps_sig, tailpool, hckpool, zxpool, xpool,
                  dram, consts):
            p.release()

    _fix_walrus_compat(nc)
    return nc


# gate permutation: Keras [i, f, g, o] -> kernel [i, f, o, g]
def _gate_perm(H):
    return np.concatenate([np.arange(0, H), np.arange(H, 2 * H),
                           np.arange(3 * H, 4 * H), np.arange(2 * H, 3 * H)])


def _pack_w(W, H):
    """[Din, G] -> [128, KD*G] bf16 with gate perm; row-major k-chunks."""
    bf = ml_dtypes.bfloat16
    Din, G = W.shape
    Wp = W[:, _gate_perm(H)]
    KD = Din // 128
    # [KD, 128, G] -> [128, KD, G]
    return np.ascontiguousarray(
        Wp.reshape(KD, 128, G).transpose(1, 0, 2).reshape(128, KD * G)
    ).astype(bf)


def _pack_b(b, H):
    MC = b.shape[0] // 128
    bp = b[_gate_perm(H)]
    return np.ascontiguousarray(bp.reshape(MC, 128).T).astype(np.float32)


def _prep_core_inputs(x, W0f, U0f, b0f, W0b, U0b, b0b,
                      W1f, U1f, b1f, W1b, U1b, b1b, T, BQ):
    """Host-side sharding: list of 8 input dicts (core = 2q + dir)."""
    bf = ml_dtypes.bfloat16
    B, _, D = x.shape
    H = U0f.shape[0]
    NT = T // CH
    KD0 = D // 128
    Wd = {0: (W0f, U0f, b0f, W1f, U1f, b1f),
          1: (W0b, U0b, b0b, W1b, U1b, b1b)}
    packed = {}
    for d in range(2):
        W0, U0, b0, W1, U1, b1 = Wd[d]
        # W1 rows: own-direction block first, partner block second
        if d == 0:
            W1o = W1
        else:
            W1o = np.concatenate([W1[H:2 * H], W1[0:H]], axis=0)
        packed[d] = {
            "w0": _pack_w(np.asarray(W0), H),
            "u0": _pack_w(np.asarray(U0), H),
            "b0": _pack_b(np.asarray(b0), H),
            "w1": _pack_w(np.asarray(W1o), H),
            "u1": _pack_w(np.asarray(U1), H),
            "b1": _pack_b(np.asarray(b1), H),
            "pflag": np.array([[1 - d]], dtype=np.int32),
        }
    in_maps = []
    for q in range(N_Q):
        xq = np.asarray(x[q * BQ:(q + 1) * BQ])      # [BQ, T, D]
        for d in range(2):
            xl = xq if d == 0 else xq[:, ::-1]
            # x_in[cc, p, kd, j*BQ+b] = xl[b, cc*CH+j, kd*128+p]
            xi = (xl.transpose(2, 1, 0)               # [D, T, BQ]
                  .reshape(KD0, 128, NT, CH, BQ)
                  .transpose(2, 1, 0, 3, 4)           # [NT, 128, KD0, CH, BQ]
                  .reshape(NT, 128, KD0 * CH * BQ))
            in_maps.append({
                "x_in": np.ascontiguousarray(xi).astype(bf),
                **packed[d],
            })
    return in_maps


def kernel(x, W0f, U0f, b0f, W0b, U0b, b0b,
           W1f, U1f, b1f, W1b, U1b, b1b):
    x = np.asarray(x, dtype=np.float32)
    B, T, D = x.shape
    H = U0f.shape[0]
    BQ = B // N_Q
    NT = T // CH
    KH = H // 128
    nc = build_program(T=T, BQ=BQ, D=D, H=H)
    in_maps = _prep_core_inputs(
        x, np.asarray(W0f), np.asarray(U0f), np.asarray(b0f),
        np.asarray(W0b), np.asarray(U0b), np.asarray(b0b),
        np.asarray(W1f), np.asarray(U1f), np.asarray(b1f),
        np.asarray(W1b), np.asarray(U1b), np.asarray(b1b), T, BQ)
    res = run_bass_kernel_spmd(nc, in_maps, list(range(N_CORES)))
    out = np.empty((B, T, 2 * H), dtype=np.float32)
    for q in range(N_Q):
        for d in range(2):
            h1 = res.results[2 * q + d]["h1out"]       # [NT, 128, KH*CH*BQ]
            h1 = h1.reshape(NT, 128, KH, CH, BQ)
            # [b, cc, j, k, p] -> [BQ, T, H]
            h1 = h1.transpose(4, 0, 3, 2, 1).reshape(BQ, T, H)
            if d == 1:
                h1 = h1[:, ::-1]
            out[q * BQ:(q + 1) * BQ, :, d * H:(d + 1) * H] = \
                h1.astype(np.float32)
    return out
